# revision 30
# baseline (speedup 1.0000x reference)
"""ChunkMHSA (banded local-window attention) Trainium2 kernel.

Full-input contract: kernel(**inputs) takes the complete tensors from
setup_inputs() and returns the full [B, T, D] output.  Internally the
sequence dimension is sharded 8 ways (256 queries per NeuronCore) with a
front/back halo of 6/3 tokens, so each core runs the whole fused pipeline
(LayerNorm -> QKV -> banded softmax(QK^T)V -> output projection ->
residual) independently -- no collectives.

Per-core dataflow (SPMD, one Bass program):
  x[tok,D] f32 --bn_stats--> mean/rstd --ts--> xr f16 --PE transpose-->
  xTr[D,tok] --PE f16 matmuls--> q,k [hk,tok] and vT [tok,hk]
  scores psum[q,s] = mask + q.k ; ACT exp(scale=1/8, accum sums) ;
  normalize on DVE ; PE transpose -> attnT[s,q] ; ctx[hk,q] = vT.T@attnT ;
  out psum[q,D] = ctx.Wo ; ACT evac ; GpSimd residual add ; DMA out.
"""

import os

os.environ.setdefault("JAX_PLATFORMS", "axon")

from contextlib import ExitStack

import numpy as np

import concourse.bass as bass
import concourse.bacc as bacc
import concourse.tile as tile
from concourse import mybir
from concourse.bass_utils import run_bass_kernel_spmd

F32 = mybir.dt.float32
F16 = mybir.dt.float16

B, T, D = 2, 2048, 512
H, DH = 8, 64
WF, WB = 6, 3
LN_EPS = 1e-3
NCORES = 8
TLOC = T // NCORES          # 256 queries per core
TIN = WF + TLOC + WB        # 265 local tokens incl. halo
NTT = 3                     # token tiles per batch (128+128+9)
NQC = 2                     # query chunks of 128 per batch
S = 128 + WF + WB           # 137 keys per query chunk
NEG = -30000.0              # additive mask value (fp16-safe)

_CACHE = {}


def _build_program():
    nc = bacc.Bacc(
        "TRN2", target_bir_lowering=False, debug=False, num_devices=NCORES
    )

    xs = nc.dram_tensor("xs", [B, TIN, D], F32, kind="ExternalInput").ap()
    wall = nc.dram_tensor("wall", [16, 128, D], F16, kind="ExternalInput").ap()
    maskd = nc.dram_tensor("maskd", [NQC, 128, S], F16, kind="ExternalInput").ap()
    eye16d = nc.dram_tensor("eye16", [128, 128], F16, kind="ExternalInput").ap()
    xq32d = nc.dram_tensor("xq32", [B, NQC, 128, D], F32, kind="ExternalInput").ap()
    outd = nc.dram_tensor("out", [B, TLOC, D], F32, kind="ExternalOutput").ap()

    with tile.TileContext(nc) as tc, ExitStack() as ctx:
        _emit(ctx, tc, xs, wall, maskd, eye16d, xq32d, outd)

    nc.compile()
    return nc


def _emit(ctx, tc, xs, wall, maskd, eye16d, xq32d, outd):
    nc = tc.nc
    EXP = mybir.ActivationFunctionType.Exp
    SQRT = mybir.ActivationFunctionType.Sqrt
    COPY = mybir.ActivationFunctionType.Copy
    SUB = mybir.AluOpType.subtract
    MULT = mybir.AluOpType.mult

    consts = ctx.enter_context(tc.tile_pool(name="consts", bufs=1))
    persist = ctx.enter_context(tc.tile_pool(name="persist", bufs=1))
    ln_tmp = ctx.enter_context(tc.tile_pool(name="ln_tmp", bufs=3))
    xr_pool = ctx.enter_context(tc.tile_pool(name="xr", bufs=3))
    attn_tmp = ctx.enter_context(tc.tile_pool(name="attn_tmp", bufs=4))
    # PSUM: 8 banks total -> tr(1) proj(2) sc(2) atm(1) att(1) ctx2(1)
    ps_tr = ctx.enter_context(tc.tile_pool(name="ps_tr", bufs=1, space="PSUM"))
    ps_proj = ctx.enter_context(tc.tile_pool(name="ps_proj", bufs=2, space="PSUM"))
    ps_sc = ctx.enter_context(tc.tile_pool(name="ps_sc", bufs=2, space="PSUM"))
    ps_at = ctx.enter_context(tc.tile_pool(name="ps_at", bufs=1, space="PSUM"))
    ps_ctx = ctx.enter_context(tc.tile_pool(name="ps_ctx", bufs=1, space="PSUM"))

    # ---- constants / weights (DMA issue spread across idle queues) ----------
    eye16 = consts.tile([128, 128], F16)
    nc.scalar.dma_start(eye16, eye16d)
    xq32 = consts.tile([128, B * NQC, D], F32)
    nc.gpsimd.dma_start(xq32, xq32d.rearrange("b c p d -> p (b c) d"))
    mask_sb = consts.tile([128, NQC, S], F16)
    nc.scalar.dma_start(mask_sb, maskd.rearrange("c p s -> p c s"))
    epst = consts.tile([128, 1], F32)
    nc.vector.memset(epst, LN_EPS)

    # all weights in one DMA: wall[4*widx + j] = chunk j of matrix widx
    w_all = consts.tile([128, 16, D], F16)
    nc.gpsimd.dma_start(w_all, wall.rearrange("m p d -> p m d"))

    def w(name, j):
        widx = "qkvo".index(name)
        return w_all[:, 4 * widx + j, :]

    # ---- x load + LayerNorm + centered/scaled xr + transpose ----------------
    x_sb = persist.tile([128, 2 * NTT, D], F32)
    xtr = persist.tile([128, 4, 2 * 384], F16)   # [dpart, dchunk, b*384+tok]
    q_sb = persist.tile([128, 4, B, TLOC], F16, tag="q_sb")
    k_sb = persist.tile([128, 4, B, TIN], F16, tag="k_sb")
    vt_sb = persist.tile([128, B, NTT, D], F16, tag="vt_sb")
    out_stage = persist.tile([128, B * NQC, D], F32, tag="out_stage")

    for b in range(B):
        nc.gpsimd.memset(x_sb[:, b * NTT + 2, :], 0.0)
    # batched x loads: one big DMA for tokens 0..255, one small tail (9 rows)
    for b in range(B):
        nc.sync.dma_start(
            x_sb[:, b * NTT : b * NTT + 2, :],
            xs[b, 0:256, :].rearrange("(i p) d -> p i d", p=128),
        )
        nc.sync.dma_start(x_sb[:9, b * NTT + 2, :], xs[b, 256:TIN, :])

    for b in range(B):
        for i in range(NTT):
            xt = x_sb[:, b * NTT + i, :]
            st = ln_tmp.tile([128, 6], F32, tag="st")
            mv = ln_tmp.tile([128, 2], F32, tag="mv")
            nc.vector.bn_stats(out=st, in_=xt)
            nc.vector.bn_aggr(out=mv, in_=st)
            sd = ln_tmp.tile([128, 1], F32, tag="sd")
            nc.scalar.activation(out=sd, in_=mv[:, 1:2], func=SQRT, bias=epst)
            rstd = ln_tmp.tile([128, 1], F32, tag="rstd")
            nc.vector.reciprocal(out=rstd, in_=sd)
            xr = xr_pool.tile([128, D], F16, tag="xr")
            nc.vector.tensor_scalar(
                out=xr, in0=xt, scalar1=mv[:, 0:1], scalar2=rstd,
                op0=SUB, op1=MULT,
            )
            pt = ps_tr.tile([128, 4, 128], F16, tag="tr")
            for j in range(4):
                nc.tensor.transpose(pt[:, j, :], xr[:, 128 * j : 128 * j + 128], eye16)
            nc.vector.tensor_copy(
                xtr[:, :, 384 * b + 128 * i : 384 * b + 128 * (i + 1)], pt
            )

    # ---- projections ---------------------------------------------------------
    # q: both batches in one matmul group (N=512), queries only
    xtr_b = xtr.rearrange("p j (b t) -> p j b t", b=2)
    for hkt in range(4):
        ps = ps_proj.tile([128, D], F32, tag="proj")
        for j in range(4):
            nc.tensor.matmul(
                ps,
                w("q", j)[:, 128 * hkt : 128 * (hkt + 1)],
                xtr_b[:, j, :, WF : WF + TLOC],
                start=(j == 0), stop=(j == 3),
            )
        nc.vector.tensor_copy(q_sb[:, hkt, :, :], ps)
    # k: per batch (keys incl. halo, N=265)
    for b in range(B):
        for hkt in range(4):
            ps = ps_proj.tile([128, D], F32, tag="proj")
            for j in range(4):
                nc.tensor.matmul(
                    ps[:, 0:TIN],
                    w("k", j)[:, 128 * hkt : 128 * (hkt + 1)],
                    xtr[:, j, 384 * b : 384 * b + TIN],
                    start=(j == 0), stop=(j == 3),
                )
            nc.scalar.activation(out=k_sb[:, hkt, b, :], in_=ps[:, 0:TIN], func=COPY)
    # vT: [tok, hk] per token tile
    for b in range(B):
        for i in range(NTT):
            ps = ps_proj.tile([128, D], F32, tag="proj")
            for j in range(4):
                nc.tensor.matmul(
                    ps,
                    xtr[:, j, 384 * b + 128 * i : 384 * b + 128 * (i + 1)],
                    w("v", j),
                    start=(j == 0), stop=(j == 3),
                )
            if i % 2 == 0:
                nc.scalar.activation(out=vt_sb[:, b, i, :], in_=ps, func=COPY)
            else:
                nc.vector.tensor_copy(vt_sb[:, b, i, :], ps)

    # ---- attention + output projection --------------------------------------
    for b in range(B):
        for cq in range(NQC):
            q0 = 128 * cq               # query index of chunk start
            s0 = 128 * cq               # local token of first key
            at_m = ps_at.tile([128, 8, 128], F16, tag="atm")   # attnT main
            at_t = ps_at.tile([9, 8, 128], F16, tag="att")     # attnT tail
            ctx2 = ps_ctx.tile([128, 4, 128], F32, tag="ctx2")
            for h in range(8):
                hp = 64 * (h % 2)
                hkt = h // 2
                sc = ps_sc.tile([128, S], F32, tag="sc")
                nc.tensor.matmul(sc, eye16, mask_sb[:, cq, :], start=True, stop=False)
                nc.tensor.matmul(
                    sc,
                    q_sb[hp : hp + 64, hkt, b, q0 : q0 + 128],
                    k_sb[hp : hp + 64, hkt, b, s0 : s0 + S],
                    start=False, stop=True,
                )
                ea = attn_tmp.tile([128, S], F16, tag="ea")
                sums = attn_tmp.tile([128, 1], F32, tag="sums")
                nc.scalar.activation(
                    out=ea, in_=sc, func=EXP, scale=0.125, accum_out=sums
                )
                rec = attn_tmp.tile([128, 1], F32, tag="rec")
                nc.vector.reciprocal(out=rec, in_=sums)
                ean = attn_tmp.tile([128, S], F16, tag="ean")
                nc.vector.tensor_scalar(
                    out=ean, in0=ea, scalar1=rec, scalar2=None, op0=MULT
                )
                nc.tensor.transpose(at_m[:, h, :], ean[:, :128], eye16)
                nc.tensor.transpose(at_t[:, h, :], ean[:, 128:S], eye16)
            atm_sb = attn_tmp.tile([128, 8, 128], F16, tag="atm_sb")
            att_sb = attn_tmp.tile([9, 8, 128], F16, tag="att_sb")
            nc.vector.tensor_copy(atm_sb, at_m)
            nc.vector.tensor_copy(att_sb, at_t)
            # ctx[hk, q]: lhsT = vT slice (LDW is 64 cols), rhs = attnT
            for h in range(8):
                hp = 64 * (h % 2)
                hkt = h // 2
                nc.tensor.matmul(
                    ctx2[hp : hp + 64, hkt, :],
                    vt_sb[:, b, cq, 64 * h : 64 * h + 64],
                    atm_sb[:, h, :],
                    start=True, stop=False,
                )
                nc.tensor.matmul(
                    ctx2[hp : hp + 64, hkt, :],
                    vt_sb[0:9, b, cq + 1, 64 * h : 64 * h + 64],
                    att_sb[0:9, h, :],
                    start=False, stop=True,
                )
            ctxn_sb = attn_tmp.tile([128, 4, 128], F16, tag="ctxn_sb")
            nc.scalar.activation(out=ctxn_sb, in_=ctx2, func=COPY)
            # out projection
            op = ps_proj.tile([128, D], F32, tag="proj")
            for j in range(4):
                nc.tensor.matmul(
                    op, ctxn_sb[:, j, :], w("o", j),
                    start=(j == 0), stop=(j == 3),
                )
            oslot = out_stage[:, b * NQC + cq, :]
            nc.scalar.activation(out=oslot, in_=op, func=COPY)
            # exact fp32 residual on the otherwise-idle GpSimd engine
            nc.gpsimd.tensor_add(oslot, oslot, xq32[:, b * NQC + cq, :])

    nc.sync.dma_start(
        outd.rearrange("b (c p) d -> p (b c) d", p=128), out_stage
    )


def _prep_host(inputs):
    """Host-side weight folding and per-core slicing."""
    x = np.asarray(inputs["x"], np.float32)
    gamma = np.asarray(inputs["gamma"], np.float32)
    beta = np.asarray(inputs["beta"], np.float32)
    Wq = np.asarray(inputs["Wq"], np.float32).reshape(D, H * DH)
    Wk = np.asarray(inputs["Wk"], np.float32).reshape(D, H * DH)
    Wv = np.asarray(inputs["Wv"], np.float32).reshape(D, H * DH)
    Wo = np.asarray(inputs["Wo"], np.float32).reshape(H * DH, D)
    bq = np.asarray(inputs["bq"], np.float32).reshape(H * DH)
    bk = np.asarray(inputs["bk"], np.float32).reshape(H * DH)
    bv = np.asarray(inputs["bv"], np.float32).reshape(H * DH)
    bo = np.asarray(inputs["bo"], np.float32).reshape(D)

    Wq2 = gamma[:, None] * Wq
    Wk2 = gamma[:, None] * Wk
    Wv2 = gamma[:, None] * Wv
    cq = bq + beta @ Wq
    ck = bk + beta @ Wk
    cv = bv + beta @ Wv
    if np.any(cq) or np.any(ck):
        raise NotImplementedError("nonzero q/k bias not supported")
    bo_eff = bo + cv @ Wo

    wall = np.concatenate(
        [
            w.reshape(4, 128, H * DH).astype(np.float16)
            for w in (Wq2, Wk2, Wv2)
        ]
        + [Wo.reshape(4, 128, D).astype(np.float16)],
        axis=0,
    )
    wall = np.ascontiguousarray(wall)

    eye16 = np.eye(128, dtype=np.float16)

    in_maps = []
    for c in range(NCORES):
        g0 = TLOC * c - WF
        xs = np.zeros((B, TIN, D), np.float32)
        lo, hi = max(0, g0), min(T, g0 + TIN)
        xs[:, lo - g0 : hi - g0, :] = x[:, lo:hi, :]

        mask = np.full((NQC, 128, S), NEG, np.float16)
        for cqi in range(NQC):
            r = np.arange(128)[:, None]
            sl = np.arange(S)[None, :]
            gj = g0 + 128 * cqi + sl
            valid = (sl - r >= 0) & (sl - r <= WF + WB) & (gj >= 0) & (gj < T)
            mask[cqi][valid] = 0.0

        xq32 = np.ascontiguousarray(
            x[:, TLOC * c : TLOC * (c + 1), :].reshape(B, NQC, 128, D)
        )
        in_maps.append(
            {
                "xs": xs, "wall": wall,
                "maskd": mask, "eye16": eye16, "xq32": xq32,
            }
        )
    return in_maps, bo_eff


def kernel(**inputs) -> np.ndarray:
    if "nc" not in _CACHE:
        _CACHE["nc"] = _build_program()
    nc = _CACHE["nc"]
    in_maps, bo_eff = _prep_host(inputs)
    res = run_bass_kernel_spmd(nc, in_maps, list(range(NCORES)))
    out = np.empty((B, T, D), np.float32)
    for c in range(NCORES):
        out[:, TLOC * c : TLOC * (c + 1), :] = res.results[c]["out"]
    if np.any(bo_eff):
        out += bo_eff
    return out


# revision 36
# speedup vs baseline: 1.3401x; 1.3401x over previous
"""ChunkMHSA (banded local-window attention) Trainium2 kernel.

Full-input contract: kernel(**inputs) takes the complete tensors from
setup_inputs() and returns the full [B, T, D] output.  Internally the
sequence dimension is sharded 8 ways (256 queries per NeuronCore) with a
front/back halo of 6/3 tokens, so each core runs the whole fused pipeline
(LayerNorm -> QKV -> banded softmax(QK^T)V -> output projection ->
residual) independently -- no collectives.

Per-core dataflow (SPMD, one Bass program):
  x[tok,D] f32 --bn_stats--> mean/rstd --ts--> xr f16 --PE transpose-->
  xTr[D,tok] --PE f16 matmuls--> q,k [hk,tok] and vT [tok,hk]
  scores psum[q,s] = mask + q.k ; ACT exp(scale=1/8, accum sums) ;
  normalize on DVE ; PE transpose -> attnT[s,q] ; ctx[hk,q] = vT.T@attnT ;
  out psum[q,D] = ctx.Wo ; ACT evac ; GpSimd residual add ; DMA out.
"""

import os

os.environ.setdefault("JAX_PLATFORMS", "axon")

from contextlib import ExitStack

import numpy as np

import concourse.bass as bass
import concourse.bacc as bacc
import concourse.tile as tile
from concourse import mybir
from concourse.bass_utils import run_bass_kernel_spmd

F32 = mybir.dt.float32
F16 = mybir.dt.float16

B, T, D = 2, 2048, 512
H, DH = 8, 64
WF, WB = 6, 3
LN_EPS = 1e-3
NCORES = 8
TLOC = T // NCORES          # 256 queries per core
TIN = WF + TLOC + WB        # 265 local tokens incl. halo
NTT = 3                     # token tiles per batch (128+128+9)
NQC = 2                     # query chunks of 128 per batch
S = 128 + WF + WB           # 137 keys per query chunk
NEG = -30000.0              # additive mask value (fp16-safe)

_CACHE = {}


def _build_program():
    nc = bacc.Bacc(
        "TRN2", target_bir_lowering=False, debug=False, num_devices=NCORES
    )

    xs = nc.dram_tensor("xs", [B, TIN, D], F32, kind="ExternalInput").ap()
    wall = nc.dram_tensor("wall", [16, 128, D], F16, kind="ExternalInput").ap()
    maskd = nc.dram_tensor("maskd", [NQC, 128, S], F16, kind="ExternalInput").ap()
    eye16d = nc.dram_tensor("eye16", [128, 128], F16, kind="ExternalInput").ap()
    xq32d = nc.dram_tensor("xq32", [B, NQC, 128, D], F32, kind="ExternalInput").ap()
    outd = nc.dram_tensor("out", [B, TLOC, D], F32, kind="ExternalOutput").ap()

    with tile.TileContext(nc) as tc, ExitStack() as ctx:
        _emit(ctx, tc, xs, wall, maskd, eye16d, xq32d, outd)

    nc.compile()
    return nc


def _emit(ctx, tc, xs, wall, maskd, eye16d, xq32d, outd):
    nc = tc.nc
    EXP = mybir.ActivationFunctionType.Exp
    SQRT = mybir.ActivationFunctionType.Sqrt
    COPY = mybir.ActivationFunctionType.Copy
    SUB = mybir.AluOpType.subtract
    MULT = mybir.AluOpType.mult

    consts = ctx.enter_context(tc.tile_pool(name="consts", bufs=1))
    persist = ctx.enter_context(tc.tile_pool(name="persist", bufs=1))
    ln_tmp = ctx.enter_context(tc.tile_pool(name="ln_tmp", bufs=3))
    xr_pool = ctx.enter_context(tc.tile_pool(name="xr", bufs=3))
    attn_tmp = ctx.enter_context(tc.tile_pool(name="attn_tmp", bufs=4))
    # PSUM: 8 banks total -> scx(3, shared by LN-transposes and scores)
    # proj(2) atm(1) att(1) ctx2(1)
    ps_scx = ctx.enter_context(tc.tile_pool(name="ps_scx", bufs=3, space="PSUM"))
    ps_proj = ctx.enter_context(tc.tile_pool(name="ps_proj", bufs=2, space="PSUM"))
    ps_at = ctx.enter_context(tc.tile_pool(name="ps_at", bufs=1, space="PSUM"))
    ps_ctx = ctx.enter_context(tc.tile_pool(name="ps_ctx", bufs=1, space="PSUM"))

    # ---- constants / weights (DMA issue spread across idle queues) ----------
    eye16 = consts.tile([128, 128], F16)
    nc.scalar.dma_start(eye16, eye16d)
    xq32 = consts.tile([128, B * NQC, D], F32)
    nc.gpsimd.dma_start(xq32, xq32d.rearrange("b c p d -> p (b c) d"))
    mask_sb = consts.tile([128, NQC, S], F16)
    nc.scalar.dma_start(mask_sb, maskd.rearrange("c p s -> p c s"))
    epst = consts.tile([128, 1], F32)
    nc.vector.memset(epst, LN_EPS)
    # warm the ACT tables (Sqrt/Exp/Copy) during the DMA prologue so the
    # ~1.5us lazy table loads don't land mid-pipeline
    warm = consts.tile([128, 1], F32)
    nc.scalar.activation(out=warm, in_=epst, func=SQRT, bias=epst)
    nc.scalar.activation(out=warm, in_=warm, func=EXP)
    nc.scalar.activation(out=warm, in_=warm, func=COPY)

    # all weights in one DMA: wall[4*widx + j] = chunk j of matrix widx
    w_all = consts.tile([128, 16, D], F16)
    nc.gpsimd.dma_start(w_all, wall.rearrange("m p d -> p m d"))

    def w(name, j):
        widx = "qkvo".index(name)
        return w_all[:, 4 * widx + j, :]

    # ---- x load + LayerNorm + centered/scaled xr + transpose ----------------
    x_sb = persist.tile([128, 2 * NTT, D], F32)
    xtr = persist.tile([128, 4, 2 * 384], F16)   # [dpart, dchunk, b*384+tok]
    q_sb = persist.tile([128, 4, B, TLOC], F16, tag="q_sb")
    k_sb = persist.tile([128, 4, B, TIN], F16, tag="k_sb")
    vt_sb = persist.tile([128, B, NTT, D], F16, tag="vt_sb")
    out_stage = persist.tile([128, B * NQC, D], F32, tag="out_stage")

    for b in range(B):
        nc.gpsimd.memset(x_sb[:, b * NTT + 2, :], 0.0)
    # per-tile x loads so LayerNorm can start on tile 0 early
    for b in range(B):
        eng = nc.sync if b == 0 else nc.scalar
        for i in range(2):
            eng.dma_start(
                x_sb[:, b * NTT + i, :], xs[b, 128 * i : 128 * (i + 1), :]
            )
        eng.dma_start(x_sb[:9, b * NTT + 2, :], xs[b, 256:TIN, :])

    for b in range(B):
        for i in range(NTT):
            xt = x_sb[:, b * NTT + i, :]
            st = ln_tmp.tile([128, 6], F32, tag="st")
            mv = ln_tmp.tile([128, 2], F32, tag="mv")
            nc.vector.bn_stats(out=st, in_=xt)
            nc.vector.bn_aggr(out=mv, in_=st)
            sd = ln_tmp.tile([128, 1], F32, tag="sd")
            nc.scalar.activation(out=sd, in_=mv[:, 1:2], func=SQRT, bias=epst)
            rstd = ln_tmp.tile([128, 1], F32, tag="rstd")
            nc.vector.reciprocal(out=rstd, in_=sd)
            xr = xr_pool.tile([128, D], F16, tag="xr")
            nc.vector.tensor_scalar(
                out=xr, in0=xt, scalar1=mv[:, 0:1], scalar2=rstd,
                op0=SUB, op1=MULT,
            )
            pt = ps_scx.tile([128, 4, 128], F16, tag="scx")
            for j in range(4):
                nc.tensor.transpose(pt[:, j, :], xr[:, 128 * j : 128 * j + 128], eye16)
            nc.vector.tensor_copy(
                xtr[:, :, 384 * b + 128 * i : 384 * b + 128 * (i + 1)], pt
            )

    # ---- projections ---------------------------------------------------------
    # q: both batches in one matmul group (N=512), queries only
    xtr_b = xtr.rearrange("p j (b t) -> p j b t", b=2)
    for hkt in range(4):
        ps = ps_proj.tile([128, D], F32, tag="proj")
        for j in range(4):
            nc.tensor.matmul(
                ps,
                w("q", j)[:, 128 * hkt : 128 * (hkt + 1)],
                xtr_b[:, j, :, WF : WF + TLOC],
                start=(j == 0), stop=(j == 3),
            )
        nc.vector.tensor_copy(q_sb[:, hkt, :, :], ps)
    # k: per batch (keys incl. halo, N=265)
    for b in range(B):
        for hkt in range(4):
            ps = ps_proj.tile([128, D], F32, tag="proj")
            for j in range(4):
                nc.tensor.matmul(
                    ps[:, 0:TIN],
                    w("k", j)[:, 128 * hkt : 128 * (hkt + 1)],
                    xtr[:, j, 384 * b : 384 * b + TIN],
                    start=(j == 0), stop=(j == 3),
                )
            nc.scalar.activation(out=k_sb[:, hkt, b, :], in_=ps[:, 0:TIN], func=COPY)
    # vT: [tok, hk] per token tile
    for b in range(B):
        for i in range(NTT):
            ps = ps_proj.tile([128, D], F32, tag="proj")
            for j in range(4):
                nc.tensor.matmul(
                    ps,
                    xtr[:, j, 384 * b + 128 * i : 384 * b + 128 * (i + 1)],
                    w("v", j),
                    start=(j == 0), stop=(j == 3),
                )
            if i % 2 == 0:
                nc.scalar.activation(out=vt_sb[:, b, i, :], in_=ps, func=COPY)
            else:
                nc.vector.tensor_copy(vt_sb[:, b, i, :], ps)

    # ---- attention + output projection --------------------------------------
    for b in range(B):
        for cq in range(NQC):
            q0 = 128 * cq               # query index of chunk start
            s0 = 128 * cq               # local token of first key
            at_m = ps_at.tile([128, 8, 128], F16, tag="atm")   # attnT main
            at_t = ps_at.tile([9, 8, 128], F16, tag="att")     # attnT tail
            ctx2 = ps_ctx.tile([128, 4, 128], F32, tag="ctx2")
            for h in range(8):
                hp = 64 * (h % 2)
                hkt = h // 2
                sc = ps_scx.tile([128, S], F32, tag="scx")
                nc.tensor.matmul(sc, eye16, mask_sb[:, cq, :], start=True, stop=False)
                nc.tensor.matmul(
                    sc,
                    q_sb[hp : hp + 64, hkt, b, q0 : q0 + 128],
                    k_sb[hp : hp + 64, hkt, b, s0 : s0 + S],
                    start=False, stop=True,
                )
                ea = attn_tmp.tile([128, S], F16, tag="ea")
                sums = attn_tmp.tile([128, 1], F32, tag="sums")
                nc.scalar.activation(
                    out=ea, in_=sc, func=EXP, scale=0.125, accum_out=sums
                )
                rec = attn_tmp.tile([128, 1], F32, tag="rec")
                nc.vector.reciprocal(out=rec, in_=sums)
                ean = attn_tmp.tile([128, S], F16, tag="ean")
                nc.vector.tensor_scalar(
                    out=ean, in0=ea, scalar1=rec, scalar2=None, op0=MULT
                )
                nc.tensor.transpose(at_m[:, h, :], ean[:, :128], eye16)
                nc.tensor.transpose(at_t[:, h, :], ean[:, 128:S], eye16)
            atm_sb = attn_tmp.tile([128, 8, 128], F16, tag="atm_sb")
            att_sb = attn_tmp.tile([9, 8, 128], F16, tag="att_sb")
            nc.vector.tensor_copy(atm_sb, at_m)
            nc.vector.tensor_copy(att_sb, at_t)
            # ctx[hk, q]: lhsT = vT slice (LDW is 64 cols), rhs = attnT
            for h in range(8):
                hp = 64 * (h % 2)
                hkt = h // 2
                nc.tensor.matmul(
                    ctx2[hp : hp + 64, hkt, :],
                    vt_sb[:, b, cq, 64 * h : 64 * h + 64],
                    atm_sb[:, h, :],
                    start=True, stop=False,
                )
                nc.tensor.matmul(
                    ctx2[hp : hp + 64, hkt, :],
                    vt_sb[0:9, b, cq + 1, 64 * h : 64 * h + 64],
                    att_sb[0:9, h, :],
                    start=False, stop=True,
                )
            ctxn_sb = attn_tmp.tile([128, 4, 128], F16, tag="ctxn_sb")
            nc.scalar.activation(out=ctxn_sb, in_=ctx2, func=COPY)
            # out projection
            op = ps_proj.tile([128, D], F32, tag="proj")
            for j in range(4):
                nc.tensor.matmul(
                    op, ctxn_sb[:, j, :], w("o", j),
                    start=(j == 0), stop=(j == 3),
                )
            oslot = out_stage[:, b * NQC + cq, :]
            nc.scalar.activation(out=oslot, in_=op, func=COPY)
            # exact fp32 residual; GpSimd for early chunks (idle engine, but
            # slow) and DVE for the last one (tail latency)
            if b * NQC + cq < B * NQC - 1:
                nc.gpsimd.tensor_add(oslot, oslot, xq32[:, b * NQC + cq, :])
            else:
                nc.vector.tensor_add(oslot, oslot, xq32[:, b * NQC + cq, :])
            nc.sync.dma_start(outd[b, 128 * cq : 128 * (cq + 1), :], oslot)


def _prep_host(inputs):
    """Host-side weight folding and per-core slicing."""
    x = np.asarray(inputs["x"], np.float32)
    gamma = np.asarray(inputs["gamma"], np.float32)
    beta = np.asarray(inputs["beta"], np.float32)
    Wq = np.asarray(inputs["Wq"], np.float32).reshape(D, H * DH)
    Wk = np.asarray(inputs["Wk"], np.float32).reshape(D, H * DH)
    Wv = np.asarray(inputs["Wv"], np.float32).reshape(D, H * DH)
    Wo = np.asarray(inputs["Wo"], np.float32).reshape(H * DH, D)
    bq = np.asarray(inputs["bq"], np.float32).reshape(H * DH)
    bk = np.asarray(inputs["bk"], np.float32).reshape(H * DH)
    bv = np.asarray(inputs["bv"], np.float32).reshape(H * DH)
    bo = np.asarray(inputs["bo"], np.float32).reshape(D)

    Wq2 = gamma[:, None] * Wq
    Wk2 = gamma[:, None] * Wk
    Wv2 = gamma[:, None] * Wv
    cq = bq + beta @ Wq
    ck = bk + beta @ Wk
    cv = bv + beta @ Wv
    if np.any(cq) or np.any(ck):
        raise NotImplementedError("nonzero q/k bias not supported")
    bo_eff = bo + cv @ Wo

    wall = np.concatenate(
        [
            w.reshape(4, 128, H * DH).astype(np.float16)
            for w in (Wq2, Wk2, Wv2)
        ]
        + [Wo.reshape(4, 128, D).astype(np.float16)],
        axis=0,
    )
    wall = np.ascontiguousarray(wall)

    eye16 = np.eye(128, dtype=np.float16)

    in_maps = []
    for c in range(NCORES):
        g0 = TLOC * c - WF
        xs = np.zeros((B, TIN, D), np.float32)
        lo, hi = max(0, g0), min(T, g0 + TIN)
        xs[:, lo - g0 : hi - g0, :] = x[:, lo:hi, :]

        mask = np.full((NQC, 128, S), NEG, np.float16)
        for cqi in range(NQC):
            r = np.arange(128)[:, None]
            sl = np.arange(S)[None, :]
            gj = g0 + 128 * cqi + sl
            valid = (sl - r >= 0) & (sl - r <= WF + WB) & (gj >= 0) & (gj < T)
            mask[cqi][valid] = 0.0

        xq32 = np.ascontiguousarray(
            x[:, TLOC * c : TLOC * (c + 1), :].reshape(B, NQC, 128, D)
        )
        in_maps.append(
            {
                "xs": xs, "wall": wall,
                "maskd": mask, "eye16": eye16, "xq32": xq32,
            }
        )
    return in_maps, bo_eff


def kernel(**inputs) -> np.ndarray:
    if "nc" not in _CACHE:
        _CACHE["nc"] = _build_program()
    nc = _CACHE["nc"]
    in_maps, bo_eff = _prep_host(inputs)
    res = run_bass_kernel_spmd(nc, in_maps, list(range(NCORES)))
    out = np.empty((B, T, D), np.float32)
    for c in range(NCORES):
        out[:, TLOC * c : TLOC * (c + 1), :] = res.results[c]["out"]
    if np.any(bo_eff):
        out += bo_eff
    return out
